# revision 1
# baseline (speedup 1.0000x reference)
"""Trainium2 Bass kernel for nn_CLS_30562987278491 (Wiener-deconvolution net).

Self-contained: hardcodes shapes B=8, NF=64, C=16, H=W=246, ks=21, FFT N=288.
Sharding: data-parallel over batch B across the 8 NeuronCores (1 image/core).

Decomposition (validated against the jax reference in fp64 numpy, rel ~1.6e-5
limited by the reference's own fp32 FFT):
  - conv_red (1x1) as matmul over channel dim.
  - 3x3 valid convs via the R=6 row-shift scheme: K=(c,dy')=128, M=(o,r)=96,
    three dx-matmuls accumulating in PSUM; leaky = 0.55*x + Abs(0.45*x).
  - adaptive pool 3x3 via a [240,3] ones-matmul + free-dim reduce.
  - FFT as DFT matmuls with the edge-replication pad FOLDED into the DFT
    matrices (Fpad [246,288]), Hermitian half-spectrum on the v axis (145),
    and the Wiener denominator |Pf|^2 computed via the 5x5 autocorrelation
    of kernel_P (so Pf itself is never materialized).
  - inverse DFT with crop (a,b in 21..266) and 1/N^2 + Hermitian weights
    folded into the inverse twiddle matrices.
  - conv_exp (1x1) as matmul.
"""
import numpy as np

import concourse.bass as bass
import concourse.bacc as bacc
import concourse.mybir as mybir
import concourse.tile as tile
from concourse.bass_utils import run_bass_kernel_spmd

import os
F32 = mybir.dt.float32
# matmul operand dtype for the big matmuls: float32r runs the PE at full rate
# (1 cyc/row at N>=256) with ~tf32 precision; float32 runs at 1/4 rate.
DT = mybir.dt.float32r if os.environ.get("KDT", "f32r") == "f32r" else mybir.dt.float32
FULLV = (DT == mybir.dt.float32r)   # F2 computes full-v spectrum (N=288 per matmul)
NP_DT = np.float32

B, NF, C, H = 8, 64, 16, 246
N = 288
VH = 145                    # N//2 + 1
KS = 21
NPIX = H * H                # 60516
CROP = 21


# ---------------------------------------------------------------- host consts
def _build_consts():
    cs = {}
    u = np.arange(N)
    v = np.arange(VH)
    F = np.exp(-2j * np.pi * np.outer(np.arange(N), u) / N)
    Fpad = np.zeros((H, N), complex)
    Fpad[0] = F[0:22].sum(0)
    Fpad[1:245] = F[22:266]
    Fpad[245] = F[266:288].sum(0)

    FuB = np.concatenate([Fpad.real, Fpad.imag], axis=1)        # [246, 576]
    cs['FuB'] = FuB.reshape(2, 123, 576).transpose(1, 0, 2)     # [123, 2, 576]

    cs['FuBn'] = (-Fpad.imag).reshape(2, 123, 288).transpose(1, 0, 2)  # [123,2,288]

    def vchunk(m):                                              # [246,145] -> [123,2,145]
        return m.reshape(2, 123, VH).transpose(1, 0, 2)
    cs['Fvr'] = vchunk(Fpad[:, :VH].real)
    cs['Fvi'] = vchunk(Fpad[:, :VH].imag)
    cs['Fvn'] = vchunk(-Fpad[:, :VH].imag)

    d5 = np.arange(5) - 2
    E5v = np.exp(-2j * np.pi * np.outer(d5, v) / N)             # [5, 145]
    cs['E5v'] = np.concatenate([E5v.real, E5v.imag], axis=1)    # [5, 290]
    th5 = 2 * np.pi * np.outer(d5, u) / N                       # [5, 288]
    cs['E5uc'] = np.cos(th5).reshape(5, 3, 96)
    cs['E5us'] = np.sin(th5).reshape(5, 3, 96)

    d21 = np.arange(21) - 10
    E21u = np.exp(-2j * np.pi * np.outer(d21, u) / N)           # [21, 288]
    cs['E21u'] = np.concatenate([E21u.real, E21u.imag], axis=1)  # [21, 576]
    E21v = np.exp(-2j * np.pi * np.outer(d21, v) / N)           # [21, 145]
    cs['E21vr'] = E21v.real.copy()
    cs['E21vi'] = E21v.imag.copy()
    cs['E21vin'] = -E21v.imag

    a = CROP + np.arange(256)
    thu = 2 * np.pi * np.outer(u, a) / N                        # [288, 256]
    cs['Eur'] = np.cos(thu).reshape(3, 96, 256).transpose(1, 0, 2)   # [96, 3, 256]
    cs['Eui'] = np.sin(thu).reshape(3, 96, 256).transpose(1, 0, 2)
    cs['Eurn'] = -cs['Eur']

    wv = np.where((v == 0) | (v == N // 2), 1.0, 2.0) / (N * N)
    bb = CROP + np.arange(256)
    thv = 2 * np.pi * np.outer(v, bb) / N                       # [145, 256]
    wEv_r = wv[:, None] * np.cos(thv)
    wEv_i = wv[:, None] * np.sin(thv)
    wEv_r[:, H:] = 0.0
    wEv_i[:, H:] = 0.0

    def vpack(m):                                               # [145,256] -> [128,2,256]
        out = np.zeros((128, 2, 256))
        out[:, 0, :] = m[:128]
        out[:17, 1, :] = m[128:]
        return out
    cs['wEvr'] = vpack(wEv_r)
    cs['wEvin'] = vpack(-wEv_i)

    rows = np.arange(240)
    pt = ((rows[:, None] // 80) == np.arange(3)[None, :]) / 6400.0   # [240, 3]
    cs['poolT'] = pt.reshape(2, 120, 3).transpose(1, 0, 2)      # [120, 2, 3]

    cs['I96'] = np.eye(96)
    return {k: np.ascontiguousarray(val, dtype=NP_DT) for k, val in cs.items()}


def _wshift(W):
    """[16,16,3,3] (o,c,dy,dx) -> [128, 3, 96]: [(c,dy'), dx, (o,r)]."""
    ws = np.zeros((128, 3, 96), NP_DT)
    for c in range(16):
        for o in range(16):
            for r in range(6):
                for dy in range(3):
                    ws[c * 8 + r + dy, :, o * 6 + r] += W[o, c, dy, :]
    return ws


CONSTS = _build_consts()
CONST_DT = {'FuB', 'FuBn', 'Fvr', 'Fvi', 'Fvn', 'Eur', 'Eui', 'Eurn', 'wEvr', 'wEvin'}


# ---------------------------------------------------------------- bass program
def _dram_ap(handle_ap, offset, dims):
    return bass.AP(tensor=handle_ap.tensor, offset=handle_ap.offset + offset, ap=[list(d) for d in dims])


def build_nc():
    nc = bacc.Bacc("TRN2", target_bir_lowering=False, debug=False)

    x_d = nc.dram_tensor("x", [NF, H, H], DT, kind="ExternalInput").ap()
    ker_d = nc.dram_tensor("ker", [21, 21], F32, kind="ExternalInput").ap()
    wredT_d = nc.dram_tensor("wredT", [64, 16], DT, kind="ExternalInput").ap()
    wg4T_d = nc.dram_tensor("wg4T", [16, 16], F32, kind="ExternalInput").ap()
    wexpT_d = nc.dram_tensor("wexpT", [16, 64], DT, kind="ExternalInput").ap()
    wsh_d = [nc.dram_tensor(f"wsh{i}", [128, 3, 96], DT, kind="ExternalInput").ap() for i in range(3)]
    cd = {}
    for k, val in CONSTS.items():
        cd[k] = nc.dram_tensor(k, list(val.shape), DT if k in CONST_DT else F32,
                               kind="ExternalInput").ap()
    y_d = nc.dram_tensor("y", [NF, H, H], F32, kind="ExternalOutput").ap()

    with tile.TileContext(nc) as tc:
        _emit(nc, tc, x_d, ker_d, wredT_d, wg4T_d, wexpT_d, wsh_d, cd, y_d)
    nc.compile()
    return nc


def _emit(nc, tc, x_d, ker_d, wredT_d, wg4T_d, wexpT_d, wsh_d, cd, y_d):
    AF = mybir.ActivationFunctionType
    OP = mybir.AluOpType

    import contextlib
    ctx = contextlib.ExitStack()
    consts = ctx.enter_context(tc.tile_pool(name="consts", bufs=1))
    singles = ctx.enter_context(tc.tile_pool(name="singles", bufs=1))
    plane = ctx.enter_context(tc.tile_pool(name="plane", bufs=3))
    convp = ctx.enter_context(tc.tile_pool(name="convp", bufs=2))
    ps = ctx.enter_context(tc.tile_pool(name="ps", bufs=8, space="PSUM"))
    dram = ctx.enter_context(tc.tile_pool(name="dram", bufs=1, space="DRAM"))

    _cp = [0]

    def copy_ps(dst, src):
        _cp[0] += 1
        if _cp[0] % 2 == 0:
            nc.vector.tensor_copy(dst, src)
        else:
            nc.scalar.activation(dst, src, AF.Copy)

    # ---- load constants
    cs = {}
    for k, ap_ in cd.items():
        t = consts.tile(list(ap_.shape), ap_.dtype, name=f"c_{k}")
        nc.sync.dma_start(t[:], ap_[:])
        cs[k] = t
    wredT = consts.tile([64, 16], DT)
    nc.sync.dma_start(wredT[:], wredT_d[:])
    wg4T = consts.tile([16, 16], F32)
    nc.sync.dma_start(wg4T[:], wg4T_d[:])
    wexpT = consts.tile([16, 64], DT)
    nc.sync.dma_start(wexpT[:], wexpT_d[:])
    wsh = []
    for i in range(3):
        t = consts.tile([128, 3, 96], DT, name=f"wsh_sb{i}")
        nc.sync.dma_start(t[:], wsh_d[i][:])
        wsh.append(t)
    kersb = consts.tile([21, 21], F32)
    nc.sync.dma_start(kersb[:], ker_d[:])

    # ---- DRAM scratch
    cls_d = dram.tile([16, 248, 246], DT)
    h1_d = dram.tile([16, 248, 244], DT)
    h2_d = dram.tile([16, 246, 242], DT)
    h3_d = dram.tile([16, 240, 240], F32)
    clear_d = dram.tile([16, 246, 246], DT)

    # zero the pad rows of cls (rows 246-247) and h1 (rows 246-247)
    zpad32 = singles.tile([16, 2, 246], F32)
    nc.vector.memset(zpad32[:], 0.0)
    zpad = singles.tile([16, 2, 246], DT)
    nc.scalar.activation(zpad[:], zpad32[:], mybir.ActivationFunctionType.Copy)
    nc.sync.dma_start(_dram_ap(cls_d, 246 * 246, [[248 * 246, 16], [246, 2], [1, 246]]),
                      zpad[:])
    nc.sync.dma_start(_dram_ap(h1_d, 246 * 244, [[248 * 244, 16], [244, 2], [1, 244]]),
                      zpad[:, :, :244])

    # ---- conv_red: cls[o, p] = sum_c wredT[c, o] * x[c, p]
    x_flat = x_d.rearrange("c h w -> c (h w)")
    cls_flat = cls_d.rearrange("o h w -> o (h w)")
    SLAB = 1024
    nslab = (NPIX + SLAB - 1) // SLAB
    for s in range(nslab):
        j0 = s * SLAB
        jn = min(SLAB, NPIX - j0)
        xs = convp.tile([64, SLAB], DT, tag="xslab")
        nc.sync.dma_start(xs[:, :jn], x_flat[:, j0:j0 + jn])
        clssb = convp.tile([16, SLAB], DT, tag="clssb")
        for j in range(0, jn, 512):
            w = min(512, jn - j)
            pt = ps.tile([16, 512], F32, tag="ps", name="ps_red")
            nc.tensor.matmul(pt[:, :w], wredT[:], xs[:, j:j + w], start=True, stop=True)
            copy_ps(clssb[:, j:j + w], pt[:, :w])
        nc.gpsimd.dma_start(cls_flat[:, j0:j0 + jn], clssb[:, :jn])

    # ---- 3x3 conv chain via R=6 row-shift
    def conv3x3(src_d, dst_d, wsh_t, R_src, W_in, W_out, nt, leaky, out_dt=DT):
        src = src_d  # [16, R_src, W_in]
        for t0 in range(0, nt, 8):
            cnt = min(8, nt - t0)
            rhs = convp.tile([128, 8, W_in], DT, tag="convrhs")
            # one DMA per dy: dst partitions {c*8+dy}, src rows {6(t0+t)+dy}
            row_sz = 8 * W_in
            for dy in range(8):
                dst = bass.AP(tensor=rhs.tensor, offset=rhs.offset + dy * row_sz,
                              ap=[[8 * row_sz, 16], [W_in, cnt], [1, W_in]])
                nc.sync.dma_start(
                    dst,
                    _dram_ap(src, (6 * t0 + dy) * W_in,
                             [[R_src * W_in, 16], [6 * W_in, cnt], [1, W_in]]))
            for tp in range(0, cnt, 2):
                c2 = min(2, cnt - tp)
                pt = ps.tile([96, 2 * W_out], F32, tag="ps", name="ps_conv")
                for dx in range(3):
                    nc.tensor.matmul(
                        pt[:, :c2 * W_out],
                        wsh_t[:, dx, :],
                        rhs[:, tp:tp + c2, dx:dx + W_out],
                        start=(dx == 0), stop=(dx == 2))
                outsb = convp.tile([96, 2, W_out], out_dt, tag="convout")
                if leaky:
                    ab = convp.tile([96, 2, W_out], F32, tag="convabs")
                    nc.scalar.activation(ab[:, :c2, :],
                                         pt[:, :c2 * W_out].rearrange("m (t j) -> m t j", t=c2),
                                         AF.Abs, scale=0.45)
                    nc.vector.scalar_tensor_tensor(
                        out=outsb[:, :c2, :],
                        in0=pt[:, :c2 * W_out].rearrange("m (t j) -> m t j", t=c2),
                        scalar=0.55, in1=ab[:, :c2, :], op0=OP.mult, op1=OP.add)
                else:
                    nc.scalar.activation(outsb[:, :c2, :],
                                         pt[:, :c2 * W_out].rearrange("m (t j) -> m t j", t=c2),
                                         AF.Copy)
                for tt in range(c2):
                    nc.gpsimd.dma_start(
                        _dram_ap(dst_d, 6 * (t0 + tp + tt) * W_out,
                                 [[dst_d.shape[1] * W_out, 16], [W_out, 6], [1, W_out]]),
                        outsb[:, tt, :])

    conv3x3(cls_d, h1_d, wsh[0], 248, 246, 244, 41, True)
    conv3x3(h1_d, h2_d, wsh[1], 248, 244, 242, 41, True)
    conv3x3(h2_d, h3_d, wsh[2], 246, 242, 240, 40, False, out_dt=F32)

    # ---- adaptive pool -> kp [16, 9]
    P1sb = singles.tile([3, 16, 240], F32)
    for cc in range(8):
        h3t = convp.tile([120, 2, 2, 240], F32, tag="h3t")
        for rc in range(2):
            nc.sync.dma_start(
                h3t[:, rc, :, :],
                _dram_ap(h3_d, cc * 2 * 240 * 240 + rc * 120 * 240,
                         [[240, 120], [240 * 240, 2], [1, 240]]))
        pt = ps.tile([3, 480], F32, tag="ps", name="ps_pool")
        for rc in range(2):
            nc.tensor.matmul(pt[:], cs['poolT'][:, rc, :],
                             h3t[:, rc, :, :].rearrange("p c w -> p (c w)"),
                             start=(rc == 0), stop=(rc == 1))
        nc.scalar.activation(P1sb[:, cc * 2:(cc + 1) * 2, :],
                             pt[:].rearrange("m (c w) -> m c w", c=2), AF.Copy)
    pooled = singles.tile([3, 16, 3], F32)
    nc.vector.tensor_reduce(pooled[:], P1sb[:].rearrange("p c (bx q) -> p c bx q", q=80),
                            axis=mybir.AxisListType.X, op=OP.add)
    pooled_c = singles.tile([16, 9], F32)
    for by in range(3):
        nc.sync.dma_start(pooled_c[:, by * 3:(by + 1) * 3], pooled[by:by + 1, :, :])

    kp = singles.tile([16, 9], F32)
    pt = ps.tile([16, 9], F32, tag="ps", name="ps_kp")
    nc.tensor.matmul(pt[:], wg4T[:], pooled_c[:], start=True, stop=True)
    ekp = singles.tile([16, 9], F32)
    nc.scalar.activation(ekp[:], pt[:], AF.Exp)
    kmean = singles.tile([16, 1], F32)
    nc.vector.tensor_reduce(kmean[:], ekp[:], axis=mybir.AxisListType.X, op=OP.add)
    kmean9 = singles.tile([16, 1], F32)
    nc.scalar.mul(kmean9[:], kmean[:], 1.0 / 9.0)
    nc.vector.tensor_scalar(out=kp[:], in0=ekp[:], scalar1=kmean9[:], scalar2=None,
                            op0=OP.subtract)

    # ---- Q autocorrelation [16, 25] then Qt [5, 16, 5]
    Q = singles.tile([16, 25], F32)
    qtmp = singles.tile([16, 9], F32)
    kp3 = kp[:].rearrange("o (r c) -> o r c", r=3)
    for dr in range(-2, 3):
        for dc in range(-2, 3):
            r0, r1 = max(0, dr), min(3, 3 + dr)
            c0, c1 = max(0, dc), min(3, 3 + dc)
            nr, ncol = r1 - r0, c1 - c0
            idx = (dr + 2) * 5 + (dc + 2)
            nc.vector.tensor_mul(qtmp[:, :nr * ncol].rearrange("o (r c) -> o r c", r=nr),
                                 kp3[:, r0:r1, c0:c1],
                                 kp3[:, r0 - dr:r1 - dr, c0 - dc:c1 - dc])
            nc.vector.tensor_reduce(Q[:, idx:idx + 1],
                                    qtmp[:, :nr * ncol],
                                    axis=mybir.AxisListType.X, op=OP.add)
    Qt = singles.tile([5, 16, 5], F32)
    Qv = Q[:].rearrange("o (dr dc) -> o dr dc", dc=5)
    for dc in range(5):
        nc.sync.dma_start(Qt[dc:dc + 1, :, :], Qv[:, :, dc])

    # ---- Kf via E21 (once per core)
    T21 = singles.tile([21, 576], F32)
    for nch in range(2):
        pt = ps.tile([21, 288], F32, tag="ps", name="ps_t21")
        nc.tensor.matmul(pt[:], kersb[:], cs['E21u'][:, nch * 288:(nch + 1) * 288],
                         start=True, stop=True)
        nc.scalar.activation(T21[:, nch * 288:(nch + 1) * 288], pt[:], AF.Copy)
    Kfr = singles.tile([96, 3, VH], F32)
    Kfi = singles.tile([96, 3, VH], F32)
    for m3 in range(3):
        ptr = ps.tile([96, VH], F32, tag="ps", name="ps_kfr")
        nc.tensor.matmul(ptr[:], T21[:, m3 * 96:(m3 + 1) * 96], cs['E21vr'][:],
                         start=True, stop=False)
        nc.tensor.matmul(ptr[:], T21[:, 288 + m3 * 96:288 + (m3 + 1) * 96], cs['E21vin'][:],
                         start=False, stop=True)
        nc.scalar.activation(Kfr[:, m3, :], ptr[:], AF.Copy)
        pti = ps.tile([96, VH], F32, tag="ps", name="ps_kfi")
        nc.tensor.matmul(pti[:], T21[:, m3 * 96:(m3 + 1) * 96], cs['E21vi'][:],
                         start=True, stop=False)
        nc.tensor.matmul(pti[:], T21[:, 288 + m3 * 96:288 + (m3 + 1) * 96], cs['E21vr'][:],
                         start=False, stop=True)
        nc.scalar.activation(Kfi[:, m3, :], pti[:], AF.Copy)
    Kf2 = singles.tile([96, 3, VH], F32)
    sqt = singles.tile([96, 3, VH], F32)
    nc.scalar.activation(Kf2[:], Kfr[:], AF.Square)
    nc.scalar.activation(sqt[:], Kfi[:], AF.Square)
    nc.vector.tensor_add(Kf2[:], Kf2[:], sqt[:])

    # ---- per-plane FFT / Wiener / IFFT
    for o in range(16):
        clsT = plane.tile([123, 2, 246], DT, tag="clsT")
        nc.sync.dma_start(clsT[:],
                          _dram_ap(cls_d, o * 248 * 246, [[246, 123], [123 * 246, 2], [1, 246]]))
        # F1: R1T[w', u] = sum_i cls[i, w'] Fpad[i, u]
        R1T = plane.tile([123, 2, 576], DT, tag="R1T")
        for m in range(2):
            for nch in range(2):
                pt = ps.tile([123, 288], F32, tag="ps", name="ps_f1")
                for k in range(2):
                    nc.tensor.matmul(pt[:], clsT[:, k, m * 123:(m + 1) * 123],
                                     cs['FuB'][:, k, nch * 288:(nch + 1) * 288],
                                     start=(k == 0), stop=(k == 1))
                copy_ps(R1T[:, m, nch * 288:(nch + 1) * 288], pt[:])
        # F2 (4-group): C[u, v].  In FULLV mode each matmul streams the full
        # 288-wide v so float32r runs at 1 cyc/row; only v<145 is copied out.
        Cr = plane.tile([96, 3, VH], F32, tag="Cr", bufs=2)
        Ci = plane.tile([96, 3, VH], F32, tag="Ci", bufs=2)
        NV = 288 if FULLV else VH
        fv_r = (lambda k: cs['FuB'][:, k, 0:288]) if FULLV else (lambda k: cs['Fvr'][:, k, :])
        fv_i = (lambda k: cs['FuB'][:, k, 288:576]) if FULLV else (lambda k: cs['Fvi'][:, k, :])
        fv_n = (lambda k: cs['FuBn'][:, k, :]) if FULLV else (lambda k: cs['Fvn'][:, k, :])
        for m3 in range(3):
            pcr = ps.tile([96, NV], F32, tag="ps", name="ps_cr")
            for k in range(2):
                nc.tensor.matmul(pcr[:], R1T[:, k, m3 * 96:(m3 + 1) * 96],
                                 fv_r(k), start=(k == 0), stop=False)
            for k in range(2):
                nc.tensor.matmul(pcr[:], R1T[:, k, 288 + m3 * 96:288 + (m3 + 1) * 96],
                                 fv_n(k), start=False, stop=(k == 1))
            copy_ps(Cr[:, m3, :], pcr[:, 0:VH])
            pci = ps.tile([96, NV], F32, tag="ps", name="ps_ci")
            for k in range(2):
                nc.tensor.matmul(pci[:], R1T[:, k, m3 * 96:(m3 + 1) * 96],
                                 fv_i(k), start=(k == 0), stop=False)
            for k in range(2):
                nc.tensor.matmul(pci[:], R1T[:, k, 288 + m3 * 96:288 + (m3 + 1) * 96],
                                 fv_r(k), start=False, stop=(k == 1))
            copy_ps(Ci[:, m3, :], pci[:, 0:VH])
        # denom = Kf2 + autocorr(kp_o) spectrum;  QE[dr, (vr|vi)] = Qt[:,o,:].T @ E5v
        pqe = ps.tile([5, 290], F32, tag="ps", name="ps_qe")
        nc.tensor.matmul(pqe[:], Qt[:, o, :], cs['E5v'][:], start=True, stop=True)
        QE = plane.tile([5, 290], F32, tag="QE", bufs=2)
        nc.scalar.activation(QE[:], pqe[:], AF.Copy)
        rec = plane.tile([96, 3, VH], F32, tag="rec", bufs=2)
        for m3 in range(3):
            pden = ps.tile([96, VH], F32, tag="ps", name="ps_den")
            nc.tensor.matmul(pden[:], cs['E5uc'][:, m3, :], QE[:, 0:VH],
                             start=True, stop=False)
            nc.tensor.matmul(pden[:], cs['E5us'][:, m3, :], QE[:, VH:290],
                             start=False, stop=False)
            nc.tensor.matmul(pden[:], cs['I96'][:], Kf2[:, m3, :],
                             start=False, stop=True)
            nc.vector.reciprocal_approx_fast(rec[:, m3, :], pden[:])
        # numerator A = C * conj(Kf);  Z = A * rec   (Zin = -Zi)
        tA = plane.tile([96, 3, VH], F32, tag="tA", bufs=2)
        tB = plane.tile([96, 3, VH], F32, tag="tB", bufs=2)
        Ar = plane.tile([96, 3, VH], F32, tag="Ar", bufs=2)
        Ain = plane.tile([96, 3, VH], F32, tag="Ain", bufs=2)
        Zr = plane.tile([96, 3, VH], DT, tag="Zr")
        Zin = plane.tile([96, 3, VH], DT, tag="Zin")
        nc.vector.tensor_mul(tA[:], Cr[:], Kfr[:])
        nc.vector.tensor_mul(tB[:], Ci[:], Kfi[:])
        nc.vector.tensor_add(Ar[:], tA[:], tB[:])
        nc.vector.tensor_mul(Zr[:], Ar[:], rec[:])
        tC = plane.tile([96, 3, VH], F32, tag="tC", bufs=2)
        tD = plane.tile([96, 3, VH], F32, tag="tD", bufs=2)
        nc.gpsimd.tensor_mul(tC[:], Cr[:], Kfi[:])
        nc.gpsimd.tensor_mul(tD[:], Ci[:], Kfr[:])
        nc.gpsimd.tensor_tensor(Ain[:], tC[:], tD[:], mybir.AluOpType.subtract)
        nc.vector.tensor_mul(Zin[:], Ain[:], rec[:])
        # I1 (4-group): HrT[v, a'], HiT[v, a']
        HrT = plane.tile([128, 2, 256], DT, tag="HrT")
        HiT = plane.tile([128, 2, 256], DT, tag="HiT")
        for m2 in range(2):
            vc = 128 if m2 == 0 else 17
            vs = slice(m2 * 128, m2 * 128 + vc)
            phr = ps.tile([128, 256], F32, tag="ps", name="ps_hr")
            for k in range(3):
                nc.tensor.matmul(phr[:vc], Zr[:, k, vs], cs['Eur'][:, k, :],
                                 start=(k == 0), stop=False)
            for k in range(3):
                nc.tensor.matmul(phr[:vc], Zin[:, k, vs], cs['Eui'][:, k, :],
                                 start=False, stop=(k == 2))
            copy_ps(HrT[:vc, m2, :], phr[:vc])
            phi = ps.tile([128, 256], F32, tag="ps", name="ps_hi")
            for k in range(3):
                nc.tensor.matmul(phi[:vc], Zr[:, k, vs], cs['Eui'][:, k, :],
                                 start=(k == 0), stop=False)
            for k in range(3):
                nc.tensor.matmul(phi[:vc], Zin[:, k, vs], cs['Eurn'][:, k, :],
                                 start=False, stop=(k == 2))
            copy_ps(HiT[:vc, m2, :], phi[:vc])
        # I2: clear[a, b]
        clearsb = plane.tile([123, 2, 246], DT, tag="clearsb")
        for ma in range(2):
            asl = slice(ma * 123, (ma + 1) * 123)
            pcl = ps.tile([123, 256], F32, tag="ps", name="ps_cl")
            nc.tensor.matmul(pcl[:], HrT[:, 0, asl], cs['wEvr'][:, 0, :],
                             start=True, stop=False)
            nc.tensor.matmul(pcl[:], HrT[:17, 1, asl], cs['wEvr'][:17, 1, :],
                             start=False, stop=False)
            nc.tensor.matmul(pcl[:], HiT[:, 0, asl], cs['wEvin'][:, 0, :],
                             start=False, stop=False)
            nc.tensor.matmul(pcl[:], HiT[:17, 1, asl], cs['wEvin'][:17, 1, :],
                             start=False, stop=True)
            copy_ps(clearsb[:, ma, :], pcl[:, 0:246])
        nc.gpsimd.dma_start(
            _dram_ap(clear_d, o * 246 * 246, [[246, 123], [123 * 246, 2], [1, 246]]),
            clearsb[:])

    # ---- conv_exp: y[e, p] = sum_o wexpT[o, e] * clear[o, p]
    clear_flat = clear_d.rearrange("o h w -> o (h w)")
    y_flat = y_d.rearrange("e h w -> e (h w)")
    for s in range(nslab):
        j0 = s * SLAB
        jn = min(SLAB, NPIX - j0)
        csl = convp.tile([16, SLAB], DT, tag="clearslab")
        nc.sync.dma_start(csl[:, :jn], clear_flat[:, j0:j0 + jn])
        ysb = convp.tile([64, SLAB], F32, tag="ysb")
        for j in range(0, jn, 512):
            w = min(512, jn - j)
            pt = ps.tile([64, 512], F32, tag="ps", name="ps_exp")
            nc.tensor.matmul(pt[:, :w], wexpT[:], csl[:, j:j + w], start=True, stop=True)
            copy_ps(ysb[:, j:j + w], pt[:, :w])
        nc.gpsimd.dma_start(y_flat[:, j0:j0 + jn], ysb[:, :jn])

    ctx.close()


_NC_CACHE = None
_LAST_RESULT = None
TRACE = False


def _get_nc():
    global _NC_CACHE
    if _NC_CACHE is None:
        _NC_CACHE = build_nc()
    return _NC_CACHE


def kernel(**inputs):
    nc = _get_nc()
    x = np.asarray(inputs['x'], np.float32)
    kerf = np.asarray(inputs['kernel'], np.float32)
    w_red = np.asarray(inputs['w_red'], np.float32)[:, :, 0, 0]     # [16, 64]
    w_g = [np.asarray(inputs[f'w_g{i}'], np.float32) for i in (1, 2, 3)]
    w_g4 = np.asarray(inputs['w_g4'], np.float32)[:, :, 0, 0]       # [16, 16]
    w_exp = np.asarray(inputs['w_exp'], np.float32)[:, :, 0, 0]     # [64, 16]

    shared = {
        'wredT': np.ascontiguousarray(w_red.T, NP_DT),
        'wg4T': np.ascontiguousarray(w_g4.T, NP_DT),
        'wexpT': np.ascontiguousarray(w_exp.T, NP_DT),
    }
    for i in range(3):
        shared[f'wsh{i}'] = _wshift(w_g[i])
    for k, val in CONSTS.items():
        shared[k] = val

    in_maps = []
    for b in range(B):
        m = dict(shared)
        m['x'] = np.ascontiguousarray(x[b], NP_DT)
        m['ker'] = np.ascontiguousarray(kerf[b, 0], NP_DT)
        in_maps.append(m)

    global _LAST_RESULT
    res = run_bass_kernel_spmd(nc, in_maps, core_ids=list(range(B)), trace=TRACE)
    _LAST_RESULT = res
    y = np.stack([res.results[b]['y'] for b in range(B)], axis=0)
    return y.astype(np.float32)

